# revision 26
# baseline (speedup 1.0000x reference)
"""Causal self-attention kernel for 8 trn2 NeuronCores.

Sharding: core c = 2*b + g handles batch b (of 4) and head-group g (of 2,
8 heads each).  Each core computes QKV projection, causal attention and the
partial output projection for its head-group; the host sums the two
head-group partials per batch (the w_proj row-split all-reduce done on host).

Matmuls run in bf16 with fp32 PSUM accumulation.  Attention is computed in
transposed orientation (S^T = K Q^T with heads-on-partitions Q/K) so softmax
needs no on-chip transposes; the softmax denominator comes free from a
ones-column appended to V (M=65 PV matmul), and the 1/rowsum broadcast runs
on the idle GPSIMD engine.
"""

import sys

if "/opt/trn_rl_repo" not in sys.path:
    sys.path.insert(0, "/opt/trn_rl_repo")

from contextlib import ExitStack

import numpy as np

import concourse.bass as bass
import concourse.mybir as mybir
import concourse.tile as tile
from concourse import bacc
from concourse.bass_utils import run_bass_kernel_spmd
from concourse.masks import make_identity

F32 = mybir.dt.float32
BF16 = mybir.dt.bfloat16
AF = mybir.ActivationFunctionType

B, T, C = 4, 2048, 1024
N_HEAD = 16
HEAD_DIM = 64
N_CORES = 8
HPC = 8          # heads per core
GC = 512         # head-group channel width (8 heads * 64)
SCALE = 0.125    # 1/sqrt(64)

_dbg = None
T_PANEL = 512    # phase-1 t panel
NTP = T // T_PANEL
NQP = T // 512   # phase-2 q panels (512 wide)


def build_program():
    nc = bacc.Bacc(
        "TRN2", target_bir_lowering=False, debug=False, num_devices=N_CORES
    )
    x_ap = nc.dram_tensor("x", [T, C], F32, kind="ExternalInput").ap()
    wq_ap = nc.dram_tensor("wq", [C, GC], F32, kind="ExternalInput").ap()
    wk_ap = nc.dram_tensor("wk", [C, GC], F32, kind="ExternalInput").ap()
    wv_ap = nc.dram_tensor("wv", [C, GC], F32, kind="ExternalInput").ap()
    wp_ap = nc.dram_tensor("wp", [GC, C], F32, kind="ExternalInput").ap()
    out_ap = nc.dram_tensor("out", [T, C], F32, kind="ExternalOutput").ap()

    with ExitStack() as ctx:
        tc = ctx.enter_context(tile.TileContext(nc))
        build_kernel(ctx, tc, x_ap, wq_ap, wk_ap, wv_ap, wp_ap, out_ap)

    nc.compile()
    return nc


def build_kernel(ctx, tc, x_ap, wq_ap, wk_ap, wv_ap, wp_ap, out_ap):
    nc = tc.nc

    # ---------------- constants ----------------
    consts = ctx.enter_context(tc.tile_pool(name="consts", bufs=1))
    ident32 = consts.tile([128, 128], F32)
    make_identity(nc, ident32)
    identb = consts.tile([128, 128], BF16)
    nc.scalar.activation(out=identb, in_=ident32, func=AF.Copy)
    onescol32 = consts.tile([128, HPC], F32)
    nc.vector.memset(onescol32, 1.0)

    # diagonal masks: dmask[i][k, q] = 1 if q >= k + off else 0 for the two
    # halves (off = 256*i, 256*i + 128) of a k256 double-block.
    dmasks = []
    for i in range(2):
        m32 = consts.tile([128, 1024], F32, tag=f"dmask32_{i}", name=f"dmask32_{i}")
        for half in range(2):
            off = 256 * i + 128 * half
            sl = m32[:, 512 * half : 512 * half + 512]
            nc.gpsimd.memset(sl, 1.0)
            nc.gpsimd.affine_select(
                out=sl,
                in_=sl,
                compare_op=mybir.AluOpType.is_ge,
                fill=0.0,
                base=-off,
                pattern=[[1, 512]],
                channel_multiplier=-1,
            )
        m = consts.tile([128, 1024], BF16, tag=f"dmask{i}", name=f"dmask{i}")
        nc.vector.tensor_copy(out=m, in_=m32)
        dmasks.append(m)

    # ---------------- persistent phase-1 outputs ----------------
    qkv_pool = ctx.enter_context(tc.tile_pool(name="qkv", bufs=1))
    QT = [qkv_pool.tile([128, T], BF16, tag=f"qt{i}", name=f"qt{i}") for i in range(4)]
    KT = [qkv_pool.tile([128, T], BF16, tag=f"kt{i}", name=f"kt{i}") for i in range(4)]
    V65 = [
        qkv_pool.tile([128, HPC * 65], BF16, tag=f"v{i}", name=f"v{i}")
        for i in range(16)
    ]
    for i in range(16):
        nc.scalar.activation(
            out=V65[i].rearrange("p (h e) -> p h e", e=65)[:, :, 64:65],
            in_=onescol32.rearrange("p (h o) -> p h o", o=1),
            func=AF.Copy,
        )

    # wp resident for phase 2 (loaded once, cast to bf16)
    wpool = ctx.enter_context(tc.tile_pool(name="w", bufs=1))
    wp_sb = []
    with tc.tile_pool(name="wstage", bufs=2) as wstage:
        for cb in range(4):
            stg = wstage.tile([128, C], F32, tag="stg")
            nc.sync.dma_start(out=stg, in_=wp_ap[128 * cb : 128 * cb + 128, :])
            t = wpool.tile([128, C], BF16, tag=f"wp{cb}", name=f"wpc{cb}")
            nc.vector.tensor_copy(out=t, in_=stg)
            wp_sb.append(t)

    # ---------------- phase 1: x^T, QT/KT, V ----------------
    with tc.tile_pool(name="wqkv", bufs=1) as wqkv_pool, \
         tc.tile_pool(name="p1sb", bufs=2) as p1sb, \
         tc.tile_pool(name="p1ps", bufs=3, space="PSUM") as p1ps, \
         tc.tile_pool(name="p1acc", bufs=3, space="PSUM") as p1acc:
        def load_x_panel(panel):
            t0 = panel * T_PANEL
            xch = []
            for ts_ in range(T_PANEL // 128):
                xt_in = p1sb.tile(
                    [128, C], F32, tag=f"x{ts_}", name=f"x{ts_}", bufs=2
                )
                nc.sync.dma_start(
                    out=xt_in,
                    in_=x_ap[t0 + 128 * ts_ : t0 + 128 * ts_ + 128, :],
                )
                xb = p1sb.tile([128, C], BF16, tag=f"xb{ts_}", name=f"xb{ts_}")
                nc.scalar.activation(out=xb, in_=xt_in, func=AF.Copy)
                xch.append(xb)
            xTp = [
                p1sb.tile([128, T_PANEL], BF16, tag=f"xT{cb}", name=f"xT{cb}")
                for cb in range(8)
            ]
            for cb in range(8):
                for ts_ in range(T_PANEL // 128):
                    pt = p1ps.tile([128, 128], BF16, tag="pt", name="pt")
                    nc.tensor.transpose(
                        pt, xch[ts_][:, 128 * cb : 128 * cb + 128], identb
                    )
                    nc.vector.tensor_copy(
                        out=xTp[cb][:, 128 * ts_ : 128 * ts_ + 128], in_=pt
                    )
            return xTp

        # x panel 0 first so PE transposes start before the 6MB weight chain
        next_xTp = load_x_panel(0)

        w_sb = {}
        for name, ap in (("wq", wq_ap), ("wk", wk_ap), ("wv", wv_ap)):
            chunks = []
            for cb in range(8):
                stg = p1sb.tile([128, GC], F32, tag="wstg", name="wstg", bufs=4)
                nc.sync.dma_start(out=stg, in_=ap[128 * cb : 128 * cb + 128, :])
                t = wqkv_pool.tile(
                    [128, GC], BF16, tag=f"{name}{cb}", name=f"{name}{cb}"
                )
                nc.vector.tensor_copy(out=t, in_=stg)
                chunks.append(t)
            w_sb[name] = chunks

        for panel in range(NTP):
            xTp = next_xTp
            t0 = panel * T_PANEL
            if panel + 1 < NTP:
                next_xTp = load_x_panel(panel + 1)
            # V: natural orientation [t128, 512], strided into V65
            for ts_ in range(T_PANEL // 128):
                acc = p1acc.tile([128, GC], F32, tag="acc")
                for cb in range(8):
                    nc.tensor.matmul(
                        acc,
                        xTp[cb][:, 128 * ts_ : 128 * ts_ + 128],
                        w_sb["wv"][cb],
                        start=(cb == 0),
                        stop=(cb == 7),
                    )
                vtile = V65[(T_PANEL // 128) * panel + ts_]
                nc.vector.tensor_copy(
                    out=vtile.rearrange("p (h e) -> p h e", e=65)[:, :, 0:64],
                    in_=acc.rearrange("p (h e) -> p h e", e=64),
                )
            # QT / KT: for each c' chunk accumulate over c
            for qk, dest in (("wq", QT), ("wk", KT)):
                for cp in range(4):
                    acc = p1acc.tile([128, T_PANEL], F32, tag="acc")
                    for cb in range(8):
                        nc.tensor.matmul(
                            acc,
                            w_sb[qk][cb][:, 128 * cp : 128 * cp + 128],
                            xTp[cb],
                            start=(cb == 0),
                            stop=(cb == 7),
                        )
                    nc.scalar.activation(
                        out=dest[cp][:, t0 : t0 + T_PANEL], in_=acc, func=AF.Copy
                    )

    # ---------------- phase 2: attention + projection ----------------
    with tc.tile_pool(name="p2ps", bufs=3, space="PSUM") as p2ps, \
         tc.tile_pool(name="p2y", bufs=1, space="PSUM") as p2y, \
         tc.tile_pool(name="p2sb", bufs=2) as p2sb, \
         tc.tile_pool(name="p2ex", bufs=3) as p2ex:

        def emit_proj(Qp, ytp, ts_list=(0, 1, 2, 3)):
            q0p = Qp * 512
            for ts_ in ts_list:
                ot = p2sb.tile([128, C], F32, tag="ot", name="ot", bufs=3)
                for co in range(2):
                    ops = p2ps.tile([128, 1024], F32, tag="sp", name="ops")
                    for cp in range(4):
                        nc.tensor.matmul(
                            ops[:, 0:512],
                            ytp[cp][:, 128 * ts_ : 128 * ts_ + 128],
                            wp_sb[cp][:, 512 * co : 512 * co + 512],
                            start=(cp == 0),
                            stop=(cp == 3),
                        )
                    nc.vector.tensor_copy(
                        out=ot[:, 512 * co : 512 * co + 512], in_=ops[:, 0:512]
                    )
                nc.sync.dma_start(
                    out=out_ap[q0p + 128 * ts_ : q0p + 128 * ts_ + 128, :], in_=ot
                )

        prev_panel = None
        for Q in range(NQP):
            q0 = Q * 512
            yt = [
                p2sb.tile([128, 512], BF16, tag=f"yt{i}", name=f"yt{i}")
                for i in range(4)
            ]
            njd = 2 * (Q + 1)
            for pair in range(4):
                ha, hb = 2 * pair, 2 * pair + 1
                ch = pair  # QT/KT chunk holding this head pair
                ypsums = [
                    p2y.tile([128, 512], F32, tag="ya", name="ya"),
                    p2y.tile([128, 512], F32, tag="yb", name="yb"),
                ]
                rows = [(0, 64), (64, 128)]
                pending = None  # (jd, [sp_a, sp_b])

                def emit_consume(jd, sps):
                    diag = jd - 2 * Q
                    for hi, h in enumerate((ha, hb)):
                        ex = p2ex.tile([128, 1024], BF16, tag=f"ex{hi}", bufs=6)
                        nc.scalar.activation(
                            out=ex, in_=sps[hi], func=AF.Exp, scale=SCALE
                        )
                        if diag >= 0:
                            nc.vector.tensor_mul(ex, ex, dmasks[diag])
                        if (
                            _dbg is not None
                            and Q == 0
                            and pair == 0
                            and jd == 0
                            and hi == 0
                        ):
                            nc.sync.dma_start(out=_dbg["ex"], in_=ex)
                        for half in range(2):
                            kb = 2 * jd + half
                            nc.tensor.matmul(
                                ypsums[hi][0:65, :],
                                V65[kb][:, 65 * h : 65 * h + 65],
                                ex[:, 512 * half : 512 * half + 512],
                                start=(jd == 0 and half == 0),
                                stop=(jd == njd - 1 and half == 1),
                            )

                for jd in range(njd):
                    sps = [
                        p2ps.tile([128, 1024], F32, tag="sp", name="spa"),
                        p2ps.tile([128, 1024], F32, tag="sp", name="spb"),
                    ]
                    # halves outer, heads inner: adjacent matmuls hit
                    # different PE row groups (base partition 0 vs 64) and
                    # run concurrently in the array.
                    for half in range(2):
                        kb = 2 * jd + half
                        for hi in range(2):
                            r0, r1 = rows[hi]
                            nc.tensor.matmul(
                                sps[hi][:, 512 * half : 512 * half + 512],
                                KT[ch][r0:r1, 128 * kb : 128 * kb + 128],
                                QT[ch][r0:r1, q0 : q0 + 512],
                                start=True,
                                stop=True,
                            )
                    if pair == 0 and jd in (1, 2) and prev_panel is not None:
                        emit_proj(*prev_panel, ts_list=(0, 1) if jd == 1 else (2, 3))
                        if jd == 2:
                            prev_panel = None
                    if pending is not None:
                        emit_consume(*pending)
                    pending = (jd, sps)
                emit_consume(*pending)

                # normalize: yt rows = ypsum[0:64] * (1/rowsum) broadcast.
                # Copy out of PSUM first (releases the ypsum bank fast),
                # then approx-reciprocal + gpsimd partition broadcast.
                for hi, h in enumerate((ha, hb)):
                    yu = p2sb.tile([64, 512], F32, tag="yu", bufs=3)
                    nc.vector.tensor_copy(out=yu, in_=ypsums[hi][0:64, :])
                    rs = p2sb.tile([1, 512], F32, tag="rs")
                    nc.vector.tensor_copy(out=rs, in_=ypsums[hi][64:65, :])
                    rec = p2sb.tile([1, 512], F32, tag="rec")
                    nc.vector.reciprocal_approx_fast(out=rec, in_=rs)
                    rb = p2sb.tile([64, 512], F32, tag="rb", bufs=3)
                    nc.gpsimd.partition_broadcast(rb, rec)
                    if _dbg is not None and Q == 0 and pair == 0 and hi == 0:
                        nc.sync.dma_start(out=_dbg["yu"][0:64, :], in_=yu)
                        nc.sync.dma_start(out=_dbg["yu"][64:65, :], in_=rs)
                        nc.sync.dma_start(out=_dbg["rec"], in_=rec)
                        nc.sync.dma_start(out=_dbg["rb"], in_=rb)
                    r0 = 64 * (h % 2)
                    nc.vector.tensor_mul(yt[h // 2][r0 : r0 + 64, :], yu, rb)

            if _dbg is not None and Q == 0:
                nc.sync.dma_start(out=_dbg["yt"], in_=yt[0])
            prev_panel = (Q, yt)
        emit_proj(*prev_panel)


_PROGRAM = None


def _get_program():
    global _PROGRAM
    if _PROGRAM is None:
        _PROGRAM = build_program()
    return _PROGRAM


def make_in_maps(x, w_qkv, w_proj):
    x = np.asarray(x, dtype=np.float32)
    w_qkv = np.asarray(w_qkv, dtype=np.float32)
    w_proj = np.asarray(w_proj, dtype=np.float32)
    in_maps = []
    for core in range(N_CORES):
        b, g = core // 2, core % 2
        c0 = GC * g
        in_maps.append(
            {
                "x": np.ascontiguousarray(x[b]),
                "wq": np.ascontiguousarray(w_qkv[:, c0 : c0 + GC]),
                "wk": np.ascontiguousarray(w_qkv[:, C + c0 : C + c0 + GC]),
                "wv": np.ascontiguousarray(w_qkv[:, 2 * C + c0 : 2 * C + c0 + GC]),
                "wp": np.ascontiguousarray(w_proj[c0 : c0 + GC, :]),
            }
        )
    return in_maps


def combine_outputs(results):
    out = np.empty((B, T, C), dtype=np.float32)
    for b in range(B):
        out[b] = results[2 * b]["out"] + results[2 * b + 1]["out"]
    return out


def kernel(x, w_qkv, w_proj):
    nc = _get_program()
    in_maps = make_in_maps(x, w_qkv, w_proj)
    res = run_bass_kernel_spmd(nc, in_maps, list(range(N_CORES)))
    return combine_outputs(res.results)


if __name__ == "__main__":
    rng = np.random.default_rng(0)
    x = rng.standard_normal((B, T, C), dtype=np.float32)
    wq = rng.standard_normal((C, 3 * C), dtype=np.float32) / 32.0
    wp = rng.standard_normal((C, C), dtype=np.float32) / 32.0
    out = kernel(x, wq, wp)
    print("ok", out.shape, float(np.abs(out).max()))


# revision 27
# speedup vs baseline: 1.0420x; 1.0420x over previous
"""Causal self-attention kernel for 8 trn2 NeuronCores.

Sharding: core c = 2*b + g handles batch b (of 4) and head-group g (of 2,
8 heads each).  Each core computes QKV projection, causal attention and the
partial output projection for its head-group; the host sums the two
head-group partials per batch (the w_proj row-split all-reduce done on host).

Matmuls run in bf16 with fp32 PSUM accumulation.  Attention is computed in
transposed orientation (S^T = K Q^T with heads-on-partitions Q/K) so softmax
needs no on-chip transposes; the softmax denominator comes free from a
ones-column appended to V (M=65 PV matmul), and the 1/rowsum broadcast runs
on the idle GPSIMD engine.
"""

import sys

if "/opt/trn_rl_repo" not in sys.path:
    sys.path.insert(0, "/opt/trn_rl_repo")

from contextlib import ExitStack

import numpy as np

import concourse.bass as bass
import concourse.mybir as mybir
import concourse.tile as tile
from concourse import bacc
from concourse.bass_utils import run_bass_kernel_spmd
from concourse.masks import make_identity

F32 = mybir.dt.float32
BF16 = mybir.dt.bfloat16
AF = mybir.ActivationFunctionType

B, T, C = 4, 2048, 1024
N_HEAD = 16
HEAD_DIM = 64
N_CORES = 8
HPC = 8          # heads per core
GC = 512         # head-group channel width (8 heads * 64)
SCALE = 0.125    # 1/sqrt(64)

_dbg = None
T_PANEL = 512    # phase-1 t panel
NTP = T // T_PANEL
NQP = T // 512   # phase-2 q panels (512 wide)


def build_program():
    nc = bacc.Bacc(
        "TRN2", target_bir_lowering=False, debug=False, num_devices=N_CORES
    )
    x_ap = nc.dram_tensor("x", [T, C], F32, kind="ExternalInput").ap()
    wq_ap = nc.dram_tensor("wq", [C, GC], F32, kind="ExternalInput").ap()
    wk_ap = nc.dram_tensor("wk", [C, GC], F32, kind="ExternalInput").ap()
    wv_ap = nc.dram_tensor("wv", [C, GC], F32, kind="ExternalInput").ap()
    wp_ap = nc.dram_tensor("wp", [GC, C], F32, kind="ExternalInput").ap()
    out_ap = nc.dram_tensor("out", [T, C], F32, kind="ExternalOutput").ap()

    with ExitStack() as ctx:
        tc = ctx.enter_context(tile.TileContext(nc))
        build_kernel(ctx, tc, x_ap, wq_ap, wk_ap, wv_ap, wp_ap, out_ap)

    nc.compile()
    return nc


def build_kernel(ctx, tc, x_ap, wq_ap, wk_ap, wv_ap, wp_ap, out_ap):
    nc = tc.nc

    # ---------------- constants ----------------
    consts = ctx.enter_context(tc.tile_pool(name="consts", bufs=1))
    ident32 = consts.tile([128, 128], F32)
    make_identity(nc, ident32)
    identb = consts.tile([128, 128], BF16)
    nc.scalar.activation(out=identb, in_=ident32, func=AF.Copy)
    onescol32 = consts.tile([128, HPC], F32)
    nc.vector.memset(onescol32, 1.0)

    # diagonal masks: dmask[i][k, q] = 1 if q >= k + off else 0 for the two
    # halves (off = 256*i, 256*i + 128) of a k256 double-block.
    dmasks = []
    for i in range(2):
        m32 = consts.tile([128, 1024], F32, tag=f"dmask32_{i}", name=f"dmask32_{i}")
        for half in range(2):
            off = 256 * i + 128 * half
            sl = m32[:, 512 * half : 512 * half + 512]
            nc.gpsimd.memset(sl, 1.0)
            nc.gpsimd.affine_select(
                out=sl,
                in_=sl,
                compare_op=mybir.AluOpType.is_ge,
                fill=0.0,
                base=-off,
                pattern=[[1, 512]],
                channel_multiplier=-1,
            )
        m = consts.tile([128, 1024], BF16, tag=f"dmask{i}", name=f"dmask{i}")
        nc.vector.tensor_copy(out=m, in_=m32)
        dmasks.append(m)

    # ---------------- persistent phase-1 outputs ----------------
    qkv_pool = ctx.enter_context(tc.tile_pool(name="qkv", bufs=1))
    QT = [qkv_pool.tile([128, T], BF16, tag=f"qt{i}", name=f"qt{i}") for i in range(4)]
    KT = [qkv_pool.tile([128, T], BF16, tag=f"kt{i}", name=f"kt{i}") for i in range(4)]
    V65 = [
        qkv_pool.tile([128, HPC * 65], BF16, tag=f"v{i}", name=f"v{i}")
        for i in range(16)
    ]
    for i in range(16):
        nc.scalar.activation(
            out=V65[i].rearrange("p (h e) -> p h e", e=65)[:, :, 64:65],
            in_=onescol32.rearrange("p (h o) -> p h o", o=1),
            func=AF.Copy,
        )

    # wp resident for phase 2 (loaded once, cast to bf16)
    wpool = ctx.enter_context(tc.tile_pool(name="w", bufs=1))
    wp_sb = []
    with tc.tile_pool(name="wstage", bufs=2) as wstage:
        for cb in range(4):
            stg = wstage.tile([128, C], F32, tag="stg")
            nc.sync.dma_start(out=stg, in_=wp_ap[128 * cb : 128 * cb + 128, :])
            t = wpool.tile([128, C], BF16, tag=f"wp{cb}", name=f"wpc{cb}")
            nc.vector.tensor_copy(out=t, in_=stg)
            wp_sb.append(t)

    # ---------------- phase 1: x^T, QT/KT, V ----------------
    with tc.tile_pool(name="wqkv", bufs=1) as wqkv_pool, \
         tc.tile_pool(name="p1sb", bufs=2) as p1sb, \
         tc.tile_pool(name="p1ps", bufs=3, space="PSUM") as p1ps, \
         tc.tile_pool(name="p1acc", bufs=3, space="PSUM") as p1acc:
        def load_x_panel(panel):
            t0 = panel * T_PANEL
            xch = []
            for ts_ in range(T_PANEL // 128):
                xt_in = p1sb.tile(
                    [128, C], F32, tag=f"x{ts_}", name=f"x{ts_}", bufs=2
                )
                nc.sync.dma_start(
                    out=xt_in,
                    in_=x_ap[t0 + 128 * ts_ : t0 + 128 * ts_ + 128, :],
                )
                xb = p1sb.tile([128, C], BF16, tag=f"xb{ts_}", name=f"xb{ts_}")
                nc.scalar.activation(out=xb, in_=xt_in, func=AF.Copy)
                xch.append(xb)
            xTp = [
                p1sb.tile([128, T_PANEL], BF16, tag=f"xT{cb}", name=f"xT{cb}")
                for cb in range(8)
            ]
            for cb in range(8):
                for ts_ in range(T_PANEL // 128):
                    pt = p1ps.tile([128, 128], BF16, tag="pt", name="pt")
                    nc.tensor.transpose(
                        pt, xch[ts_][:, 128 * cb : 128 * cb + 128], identb
                    )
                    nc.vector.tensor_copy(
                        out=xTp[cb][:, 128 * ts_ : 128 * ts_ + 128], in_=pt
                    )
            return xTp

        # x panel 0 first so PE transposes start before the 6MB weight chain
        next_xTp = load_x_panel(0)

        w_sb = {}
        for name, ap in (("wq", wq_ap), ("wk", wk_ap), ("wv", wv_ap)):
            chunks = []
            for cb in range(8):
                stg = p1sb.tile([128, GC], F32, tag="wstg", name="wstg", bufs=4)
                nc.sync.dma_start(out=stg, in_=ap[128 * cb : 128 * cb + 128, :])
                t = wqkv_pool.tile(
                    [128, GC], BF16, tag=f"{name}{cb}", name=f"{name}{cb}"
                )
                nc.vector.tensor_copy(out=t, in_=stg)
                chunks.append(t)
            w_sb[name] = chunks

        for panel in range(NTP):
            xTp = next_xTp
            t0 = panel * T_PANEL
            if panel + 1 < NTP:
                next_xTp = load_x_panel(panel + 1)
            # QT / KT: for each c' chunk accumulate over c
            for qk, dest in (("wq", QT), ("wk", KT)):
                for cp in range(4):
                    acc = p1acc.tile([128, T_PANEL], F32, tag="acc")
                    for cb in range(8):
                        nc.tensor.matmul(
                            acc,
                            w_sb[qk][cb][:, 128 * cp : 128 * cp + 128],
                            xTp[cb],
                            start=(cb == 0),
                            stop=(cb == 7),
                        )
                    nc.scalar.activation(
                        out=dest[cp][:, t0 : t0 + T_PANEL], in_=acc, func=AF.Copy
                    )
            # V: natural orientation [t128, 512], strided into V65
            for ts_ in range(T_PANEL // 128):
                acc = p1acc.tile([128, GC], F32, tag="acc")
                for cb in range(8):
                    nc.tensor.matmul(
                        acc,
                        xTp[cb][:, 128 * ts_ : 128 * ts_ + 128],
                        w_sb["wv"][cb],
                        start=(cb == 0),
                        stop=(cb == 7),
                    )
                vtile = V65[(T_PANEL // 128) * panel + ts_]
                nc.vector.tensor_copy(
                    out=vtile.rearrange("p (h e) -> p h e", e=65)[:, :, 0:64],
                    in_=acc.rearrange("p (h e) -> p h e", e=64),
                )

    # ---------------- phase 2: attention + projection ----------------
    with tc.tile_pool(name="p2ps", bufs=3, space="PSUM") as p2ps, \
         tc.tile_pool(name="p2y", bufs=1, space="PSUM") as p2y, \
         tc.tile_pool(name="p2sb", bufs=2) as p2sb, \
         tc.tile_pool(name="p2ex", bufs=3) as p2ex:

        def emit_proj(Qp, ytp, ts_list=(0, 1, 2, 3)):
            q0p = Qp * 512
            for ts_ in ts_list:
                ot = p2sb.tile([128, C], F32, tag="ot", name="ot", bufs=3)
                for co in range(2):
                    ops = p2ps.tile([128, 1024], F32, tag="sp", name="ops")
                    for cp in range(4):
                        nc.tensor.matmul(
                            ops[:, 0:512],
                            ytp[cp][:, 128 * ts_ : 128 * ts_ + 128],
                            wp_sb[cp][:, 512 * co : 512 * co + 512],
                            start=(cp == 0),
                            stop=(cp == 3),
                        )
                    nc.vector.tensor_copy(
                        out=ot[:, 512 * co : 512 * co + 512], in_=ops[:, 0:512]
                    )
                nc.sync.dma_start(
                    out=out_ap[q0p + 128 * ts_ : q0p + 128 * ts_ + 128, :], in_=ot
                )

        prev_panel = None
        for Q in range(NQP):
            q0 = Q * 512
            yt = [
                p2sb.tile([128, 512], BF16, tag=f"yt{i}", name=f"yt{i}")
                for i in range(4)
            ]
            njd = 2 * (Q + 1)
            for pair in range(4):
                ha, hb = 2 * pair, 2 * pair + 1
                ch = pair  # QT/KT chunk holding this head pair
                ypsums = [
                    p2y.tile([128, 512], F32, tag="ya", name="ya"),
                    p2y.tile([128, 512], F32, tag="yb", name="yb"),
                ]
                rows = [(0, 64), (64, 128)]
                pending = None  # (jd, [sp_a, sp_b])

                def emit_consume(jd, sps):
                    diag = jd - 2 * Q
                    for hi, h in enumerate((ha, hb)):
                        ex = p2ex.tile([128, 1024], BF16, tag=f"ex{hi}", bufs=6)
                        nc.scalar.activation(
                            out=ex, in_=sps[hi], func=AF.Exp, scale=SCALE
                        )
                        if diag >= 0:
                            nc.vector.tensor_mul(ex, ex, dmasks[diag])
                        if (
                            _dbg is not None
                            and Q == 0
                            and pair == 0
                            and jd == 0
                            and hi == 0
                        ):
                            nc.sync.dma_start(out=_dbg["ex"], in_=ex)
                        for half in range(2):
                            kb = 2 * jd + half
                            nc.tensor.matmul(
                                ypsums[hi][0:65, :],
                                V65[kb][:, 65 * h : 65 * h + 65],
                                ex[:, 512 * half : 512 * half + 512],
                                start=(jd == 0 and half == 0),
                                stop=(jd == njd - 1 and half == 1),
                            )

                for jd in range(njd):
                    sps = [
                        p2ps.tile([128, 1024], F32, tag="sp", name="spa"),
                        p2ps.tile([128, 1024], F32, tag="sp", name="spb"),
                    ]
                    # halves outer, heads inner: adjacent matmuls hit
                    # different PE row groups (base partition 0 vs 64) and
                    # run concurrently in the array.
                    for half in range(2):
                        kb = 2 * jd + half
                        for hi in range(2):
                            r0, r1 = rows[hi]
                            nc.tensor.matmul(
                                sps[hi][:, 512 * half : 512 * half + 512],
                                KT[ch][r0:r1, 128 * kb : 128 * kb + 128],
                                QT[ch][r0:r1, q0 : q0 + 512],
                                start=True,
                                stop=True,
                            )
                    if pair == 0 and jd in (1, 2) and prev_panel is not None:
                        emit_proj(*prev_panel, ts_list=(0, 1) if jd == 1 else (2, 3))
                        if jd == 2:
                            prev_panel = None
                    if pending is not None:
                        emit_consume(*pending)
                    pending = (jd, sps)
                emit_consume(*pending)

                # normalize: yt rows = ypsum[0:64] * (1/rowsum) broadcast.
                # Copy out of PSUM first (releases the ypsum bank fast),
                # then approx-reciprocal + gpsimd partition broadcast.
                for hi, h in enumerate((ha, hb)):
                    yu = p2sb.tile([64, 512], F32, tag="yu", bufs=3)
                    nc.vector.tensor_copy(out=yu, in_=ypsums[hi][0:64, :])
                    rs = p2sb.tile([1, 512], F32, tag="rs")
                    nc.vector.tensor_copy(out=rs, in_=ypsums[hi][64:65, :])
                    rec = p2sb.tile([1, 512], F32, tag="rec")
                    nc.vector.reciprocal_approx_fast(out=rec, in_=rs)
                    rb = p2sb.tile([64, 512], F32, tag="rb", bufs=3)
                    nc.gpsimd.partition_broadcast(rb, rec)
                    if _dbg is not None and Q == 0 and pair == 0 and hi == 0:
                        nc.sync.dma_start(out=_dbg["yu"][0:64, :], in_=yu)
                        nc.sync.dma_start(out=_dbg["yu"][64:65, :], in_=rs)
                        nc.sync.dma_start(out=_dbg["rec"], in_=rec)
                        nc.sync.dma_start(out=_dbg["rb"], in_=rb)
                    r0 = 64 * (h % 2)
                    nc.vector.tensor_mul(yt[h // 2][r0 : r0 + 64, :], yu, rb)

            if _dbg is not None and Q == 0:
                nc.sync.dma_start(out=_dbg["yt"], in_=yt[0])
            prev_panel = (Q, yt)
        emit_proj(*prev_panel)


_PROGRAM = None


def _get_program():
    global _PROGRAM
    if _PROGRAM is None:
        _PROGRAM = build_program()
    return _PROGRAM


def make_in_maps(x, w_qkv, w_proj):
    x = np.asarray(x, dtype=np.float32)
    w_qkv = np.asarray(w_qkv, dtype=np.float32)
    w_proj = np.asarray(w_proj, dtype=np.float32)
    in_maps = []
    for core in range(N_CORES):
        b, g = core // 2, core % 2
        c0 = GC * g
        in_maps.append(
            {
                "x": np.ascontiguousarray(x[b]),
                "wq": np.ascontiguousarray(w_qkv[:, c0 : c0 + GC]),
                "wk": np.ascontiguousarray(w_qkv[:, C + c0 : C + c0 + GC]),
                "wv": np.ascontiguousarray(w_qkv[:, 2 * C + c0 : 2 * C + c0 + GC]),
                "wp": np.ascontiguousarray(w_proj[c0 : c0 + GC, :]),
            }
        )
    return in_maps


def combine_outputs(results):
    out = np.empty((B, T, C), dtype=np.float32)
    for b in range(B):
        out[b] = results[2 * b]["out"] + results[2 * b + 1]["out"]
    return out


def kernel(x, w_qkv, w_proj):
    nc = _get_program()
    in_maps = make_in_maps(x, w_qkv, w_proj)
    res = run_bass_kernel_spmd(nc, in_maps, list(range(N_CORES)))
    return combine_outputs(res.results)


if __name__ == "__main__":
    rng = np.random.default_rng(0)
    x = rng.standard_normal((B, T, C), dtype=np.float32)
    wq = rng.standard_normal((C, 3 * C), dtype=np.float32) / 32.0
    wp = rng.standard_normal((C, C), dtype=np.float32) / 32.0
    out = kernel(x, wq, wp)
    print("ok", out.shape, float(np.abs(out).max()))
